# revision 4
# baseline (speedup 1.0000x reference)
"""Trainium2 Bass kernel for BlockAttentionResidual (fp16 staging).

Reference computation (fp32):
    K      = rmsnorm(V, w)                      # over d
    logits = einsum('d,lbtd->lbt', q, K)
    attn   = softmax(logits, axis=l)
    h      = einsum('lbt,lbtd->btd', attn, V)

Design (per core; tokens = flattened (b,t) sharded 8 ways, 16 tiles of 128):
  - V is cast to fp16 on the host and staged tile-major [NT, 128, L, D]:
    one contiguous 2 MiB DMA per token tile, halving HBM traffic vs fp32.
  - ssq (8/tile) on ACT: Square with accum_out into a stride-0 broadcast
    sink; all ACT functions (Square/Ln/Exp/Copy) pinned to the single
    natural_log_exp_and_others table set (no per-tile table reloads).
  - dots (8/tile) on DVE: fused scalar_tensor_tensor multiply+accumulate
    against a host-broadcast q*w row (fused DVE reduces run 1x; plain TT
    2x does not have an accumulator, so stt is optimal per-op).
  - inv = exp(-0.5*ln(mean+eps)) on ACT; softmax smalls split ACT/DVE.
  - h: all 8 l-slices accumulate on the PE as diag(e_l) @ V_l in PSUM
    (fp16 matmuls, 1 cycle/row); 1/s folds into the ACT PSUM->SBUF drain;
    h stored fp16 and upcast on the host (rel err ~1e-3, gate is 2e-2).
  - software pipeline: tile i's recip/diags/matmuls/drain/store are
    emitted under tile i+1's square/dot streams; stores issue on the ACT
    HWDGE ring so they never block loads on the sync ring.
Measured: ~210 us/pass vs 965 us baseline (fp32, thrashing ACT tables).
"""

from contextlib import ExitStack

import numpy as np

import concourse.bass as bass
import concourse.mybir as mybir
import concourse.tile as tile
from concourse import bacc
from concourse.bass_utils import run_bass_kernel_spmd

NCORES = 8
L = 8
B = 4
T = 4096
D = 1024
BT = B * T
TOK = BT // NCORES  # tokens per core
P = 128
NT = TOK // P  # token tiles per core
HALF = 512  # PSUM-bank limit on a single matmul's output (fp32)
EPS = 1e-6
F32 = mybir.dt.float32
F16 = mybir.dt.float16

_CACHE: dict = {}

import os as _os


def _pin_act_tables():
    """Route every ACT function through `natural_log_exp_and_others` (which
    holds Square, Ln, Exp, Copy) so the table-load pass emits one hoisted
    load instead of alternating natural_log <-> exp_and_others every tile
    (~2 x 2.7us per tile on HW)."""
    import concourse.bacc as _bacc_mod
    from concourse.hw_specs import get_activation_tables as _orig

    if getattr(_bacc_mod.get_activation_tables, "_pinned", False):
        return
    PIN = "natural_log_exp_and_others"

    def patched(arch):
        tabs = _orig(arch)
        pinned = tabs[PIN]
        return {
            name: (fns if name == PIN else (fns - pinned))
            for name, fns in tabs.items()
        }

    patched._pinned = True
    _bacc_mod.get_activation_tables = patched

K_SQ_ACT = int(_os.environ.get("K_SQ_ACT", "8"))  # squares on ACT (rest DVE)
K_VBUFS = int(_os.environ.get("K_VBUFS", "5"))
K_PBUFS = int(_os.environ.get("K_PBUFS", "2"))  # [P,2,D] pair tiles = 4 banks
K_STORE = _os.environ.get("K_STORE", "scalar")  # engine for h stores
K_PIPE = int(_os.environ.get("K_PIPE", "1"))  # delay diag+mm by one tile
K_DIAG = _os.environ.get("K_DIAG", "dve")  # engine for diag builds (dve|act|pool)
K_PDOTS = int(_os.environ.get("K_PDOTS", "0"))  # dots offloaded to gpsimd
K_DACT = int(_os.environ.get("K_DACT", "0"))  # diags on ACT (rest per K_DIAG)


def _build_nc(sq_act=None, mode="full", reps=1, vbufs=None, pbufs=None,
              store_eng=None, pipe=None, nt=NT, diag_eng=None, pool_dots=None,
              d_act=None):
    sq_act = K_SQ_ACT if sq_act is None else sq_act
    vbufs = K_VBUFS if vbufs is None else vbufs
    pbufs = K_PBUFS if pbufs is None else pbufs
    store_eng = K_STORE if store_eng is None else store_eng
    pipe = K_PIPE if pipe is None else pipe
    diag_eng = K_DIAG if diag_eng is None else diag_eng
    pool_dots = K_PDOTS if pool_dots is None else pool_dots
    d_act = K_DACT if d_act is None else d_act
    _pin_act_tables()
    A = mybir.ActivationFunctionType
    O = mybir.AluOpType
    X = mybir.AxisListType.X

    nc = bacc.Bacc(
        "TRN2",
        target_bir_lowering=False,
        debug=False,
        enable_asserts=False,
        num_devices=NCORES,
    )
    v_d = nc.dram_tensor("v", [NT, P, L, D], F16, kind="ExternalInput")
    qwb_d = nc.dram_tensor("qwb", [P, D], F16, kind="ExternalInput")
    id_d = nc.dram_tensor("ident", [P, P], F16, kind="ExternalInput")
    h_d = nc.dram_tensor("h", [TOK, D], F16, kind="ExternalOutput")

    with tile.TileContext(nc) as tc, ExitStack() as ctx:
        cpool = ctx.enter_context(tc.tile_pool(name="const", bufs=1))
        vpool = ctx.enter_context(tc.tile_pool(name="vin", bufs=vbufs))
        spool = ctx.enter_context(tc.tile_pool(name="small", bufs=4))
        jpool = ctx.enter_context(tc.tile_pool(name="scratch", bufs=1))
        dpool = ctx.enter_context(tc.tile_pool(name="diag", bufs=8))
        hpool = ctx.enter_context(tc.tile_pool(name="hout", bufs=3))
        ppool = ctx.enter_context(
            tc.tile_pool(name="psum", bufs=pbufs, space=bass.MemorySpace.PSUM)
        )

        qwb = cpool.tile([P, D], F16, tag="qwb")
        ident = cpool.tile([P, P], F16, tag="ident")
        nc.sync.dma_start(qwb[:], qwb_d[:])
        nc.sync.dma_start(ident[:], id_d[:])

        # stride-0 sinks for the full-size primary outputs of the fused
        # reduce ops (only the accum_out is consumed)
        jact = jpool.tile([P, 1], F32, tag="jact")
        jact_out = jact.broadcast_to((P, D))
        jvec = jpool.tile([P, D], F16, tag="jvec")
        jvecp = jpool.tile([P, D], F16, tag="jvecp")  # gpsimd-private sink

        # per-partition bias constants
        zero_b = cpool.tile([P, 1], F32, tag="zero_b")
        eps_b = cpool.tile([P, 1], F32, tag="eps_b")
        nc.vector.memset(zero_b[:], 0.0)
        nc.vector.memset(eps_b[:], EPS)

        store = getattr(nc, store_eng)
        diag_e = nc.gpsimd if diag_eng == "pool" else nc.vector

        def emit_recip(st):
            r = spool.tile([P, 1], F32, tag="r")
            nc.vector.reciprocal(r[:], st["s"][:])
            st["r"] = r

        pair_state = {"hp": None}

        def emit_diag_mm(st):
            # h = sum_l (e_l/s) * V_l via diag(e_l*r) matmuls in PSUM; the
            # softmax 1/s folds into the diag build (dual-scalar tensor_scalar)
            # so the pair drain below is a pure copy. Tiles pair up into one
            # [P, 2, D] PSUM tile (4 banks) drained + stored together.
            half = st["i"] % 2
            if half == 0 or pair_state["hp"] is None:
                pair_state["hp"] = ppool.tile([P, 2, D], F32, tag="hp", name="hp")
            hp = pair_state["hp"]
            st["hp"] = hp
            st["half"] = half
            for l in range(L):
                dg = dpool.tile([P, P], F16, tag="dg")
                if l < d_act or diag_eng == "act":
                    nc.scalar.mul(dg[:], ident[:], st["e"][:, l : l + 1])
                else:
                    nc.vector.tensor_scalar(
                        dg[:], ident[:], st["e"][:, l : l + 1], st["r"][:],
                        O.mult, O.mult,
                    )
                for h_ in range(2):
                    nc.tensor.matmul(
                        hp[:, half, h_ * HALF : (h_ + 1) * HALF],
                        dg[:],
                        st["vt"][l][:, h_ * HALF : (h_ + 1) * HALF],
                        start=(l == 0),
                        stop=(l == L - 1),
                    )

        def emit_drain(st):
            # drain only when this tile completes a pair (odd half) or at tail
            if st["half"] == 0:
                hs = hpool.tile([P, D], F16, tag="hs1")
                nc.scalar.copy(hs[:], st["hp"][:, 0, :])
                store.dma_start(h_d[st["i"] * P : (st["i"] + 1) * P, :], hs[:])
                return
            i0 = st["i"] - 1
            hs2 = hpool.tile([P, 2, D], F16, tag="hs2")
            nc.scalar.copy(hs2[:], st["hp"][:])
            store.dma_start(
                h_d[i0 * P : (i0 + 2) * P, :].rearrange("(n p) d -> p n d", n=2),
                hs2[:],
            )

        pend = None  # previous tile: needs recip, diag+mm, drain
        for rep_i in range(reps * nt):
            i = rep_i % nt
            vta = vpool.tile([P, L, D], F16, tag="vta", name="vta")
            nc.sync.dma_start(vta[:], v_d[i])
            vt = [vta[:, l, :] for l in range(L)]

            if mode == "dmaonly":
                hs = hpool.tile([P, D], F16, tag="hs")
                nc.vector.tensor_copy(hs[:], vt[0][:])
                store.dma_start(h_d[i * P : (i + 1) * P, :], hs[:])
                continue

            if pend is not None:
                # prev tile's tail work, scheduled under this tile's streams:
                # DVE: recip + diags first (PE starts early), ACT drain later
                emit_recip(pend)
                emit_diag_mm(pend)

            ssq = spool.tile([P, L], F32, tag="ssq")
            dotv = spool.tile([P, L], F32, tag="dotv")
            for l in range(L):
                if l < sq_act:
                    nc.scalar.activation(
                        jact_out,
                        vt[l][:],
                        A.Square,
                        bias=zero_b[:],
                        accum_out=ssq[:, l : l + 1],
                    )
                else:
                    nc.vector.scalar_tensor_tensor(
                        jvec[:], vt[l][:], 1.0, vt[l][:], O.mult, O.mult,
                        accum_out=ssq[:, l : l + 1],
                    )
                if l >= L - pool_dots:
                    nc.gpsimd.scalar_tensor_tensor(
                        jvecp[:], vt[l][:], 1.0, qwb[:], O.mult, O.mult,
                        accum_out=dotv[:, l : l + 1],
                    )
                else:
                    nc.vector.scalar_tensor_tensor(
                        jvec[:], vt[l][:], 1.0, qwb[:], O.mult, O.mult,
                        accum_out=dotv[:, l : l + 1],
                    )

            # inv = rsqrt(mean + eps) = exp(-0.5 * ln(ssq/D + eps))
            lnm = spool.tile([P, L], F32, tag="lnm")
            nc.scalar.activation(lnm[:], ssq[:], A.Ln, scale=1.0 / D, bias=eps_b[:])
            inv = spool.tile([P, L], F32, tag="inv")
            nc.scalar.activation(inv[:], lnm[:], A.Exp, scale=-0.5, bias=zero_b[:])

            # ACT does the prev pair's PSUM drain here — it fills the gap
            # while DVE finishes this tile's dots and the softmax smalls
            if pend is not None:
                if pend["half"] == 1:
                    emit_drain(pend)
                pend = None

            logits = spool.tile([P, L], F32, tag="logits")
            nc.vector.tensor_mul(logits[:], dotv[:], inv[:])
            nm = spool.tile([P, 1], F32, tag="nm")
            nc.vector.tensor_reduce(nm[:], logits[:], X, O.max, negate=True)
            e = spool.tile([P, L], F32, tag="e")
            s = spool.tile([P, 1], F32, tag="s")
            nc.scalar.activation(e[:], logits[:], A.Exp, bias=nm[:], accum_out=s[:])

            st = {"i": i, "vt": vt, "e": e, "s": s}
            if pipe:
                pend = st
            else:
                emit_recip(st)
                emit_diag_mm(st)
                emit_drain(st)

        if mode == "full" and pend is not None:
            emit_recip(pend)
            emit_diag_mm(pend)
            emit_drain(pend)

    nc.compile()
    return nc


def get_nc():
    if "nc" not in _CACHE:
        _CACHE["nc"] = _build_nc()
    return _CACHE["nc"]


def build_variant(**kw):
    return _build_nc(**kw)


def make_in_maps(blocks, query, norm_weight):
    qw = (
        np.asarray(query, np.float32) * np.asarray(norm_weight, np.float32)
    ).astype(np.float16)
    qwb = np.ascontiguousarray(np.broadcast_to(qw, (P, D)))
    ident = np.eye(P, dtype=np.float16)
    vr = np.asarray(blocks, np.float32).reshape(L, BT, D)
    in_maps = []
    for c in range(NCORES):
        chunk = vr[:, c * TOK : (c + 1) * TOK, :].astype(np.float16)  # [L,TOK,D]
        chunk = chunk.transpose(1, 0, 2).reshape(NT, P, L, D)  # [NT,128,L,D]
        in_maps.append(
            {
                "v": np.ascontiguousarray(chunk),
                "qwb": qwb,
                "ident": ident,
            }
        )
    return in_maps


def kernel(blocks, query, norm_weight):
    import time

    nc = get_nc()
    in_maps = make_in_maps(blocks, query, norm_weight)
    last_exc = None
    for attempt in range(3):
        try:
            res = run_bass_kernel_spmd(nc, in_maps, core_ids=list(range(NCORES)))
            break
        except Exception as exc:  # transient device-wedge after a prior crash
            last_exc = exc
            time.sleep(45)
    else:
        raise last_exc
    h = np.concatenate(
        [res.results[c]["h"].astype(np.float32) for c in range(NCORES)], axis=0
    )
    return h.reshape(B, T, D)
